# revision 10
# baseline (speedup 1.0000x reference)
"""Trainium2 Bass kernel for the AdvancedMambaAMT block (sparse attention).

Strategy:
  - Data-parallel over batch: 16 batches / 8 cores = 2 batches per core.
    No collectives needed; host shards inputs and concatenates outputs.
  - The causal*decay mask, under XLA's flush-to-zero float32 semantics,
    is exactly: allowed(i,j) = (j <= i) AND (i + j <= 873).
    Queries i >= 874 have NO allowed keys -> reference produces NaN rows;
    we reproduce those NaNs host-side.  Keys j >= 437 are never attended.
  - All activations are kept feature-major [feature_chunk(128), token] so
    the whole matmul chain needs zero on-device transposes; `x` is
    pre-transposed on the host.  LayerNorm statistics (partition-axis
    reductions) are computed with ones-vector matmuls on the PE, and
    per-token scalars are broadcast across partitions with K=1 matmuls.
  - Attention is computed in [key, query] orientation: scoresT = k^T q,
    p = exp(scale*scoresT) * mask01, den = ones^T p, out = v^T (p/den).
    Only the 4 non-empty key chunks and their exact query extents are
    computed (~18% of the dense score matrix).
  - Matmul operands are bf16 (1 PE cycle/row vs fp32's 4); accumulation
    stays fp32 in PSUM, softmax/LN statistics stay fp32.
"""

import os
import sys

os.environ.setdefault("MYCRO_LOCAL_CACHE", "1")
if "/opt/trn_rl_repo" not in sys.path:
    sys.path.insert(0, "/opt/trn_rl_repo")

import ml_dtypes
import numpy as np

import concourse.bass as bass
import concourse.mybir as mybir
import concourse.tile as tile
from concourse import bacc
from concourse.bass_utils import run_bass_kernel_spmd

F32 = mybir.dt.float32
BF16 = mybir.dt.bfloat16
AF = mybir.ActivationFunctionType
OP = mybir.AluOpType
NPBF = ml_dtypes.bfloat16

B, N, D = 16, 1024, 512
H, HD, OUT = 4, 128, 88
DH = 2 * D          # FE hidden 1024
DF = D // 2         # fc1 out 256
EPS = 1e-5
SCALE = 1.0 / float(np.sqrt(HD))

THR = 873           # allowed(i,j) = (j<=i) & (i+j<=THR); verified vs jax in test.py
NT = THR + 1        # 874 valid query tokens
NTP = 896           # 7*128, padded token count we compute
NK = 512            # key tokens used (4 chunks; real max key is 436)
NCORES = 8
BPC = B // NCORES   # batches per core

PIECES = [(0, 448), (448, NTP)]          # column pieces (PSUM-bank sized)
NC_TOK = NTP // 128                      # 7 token chunks
NKC = NK // 128                          # 4 key chunks


# ---------------------------------------------------------------- host mask --
def _mask_meta():
    """Slab extents, mask-multiply schedule, and packed mask tiles.

    Returns (slabs, mask_apps, mask_np):
      slabs[J] = (lo, hi) query-column range computed for key chunk J
      mask_apps[J] = list of (tile_idx, a, b): p[:, a-lo:b-lo] *= mask[ti][:, :b-a]
      mask_np = [n_tiles, 128, 128] float32 0/1 tiles in [j, i] orientation
    """
    i = np.arange(N)[:, None]
    j = np.arange(N)[None, :]
    allowed = ((j <= i) & (i + j <= THR)).astype(np.float32)  # [i, j]
    slabs = []
    for J in range(NKC):
        lo = 128 * J
        hi = THR - 128 * J + 1
        if J == 0:
            hi = NTP                      # force full PSUM-column initialization
        slabs.append((lo, hi))
    tiles = []
    mask_apps = [[] for _ in range(NKC)]
    for J in range(NKC):
        lo, hi = slabs[J]
        for I in range((NTP + 127) // 128):
            a = max(lo, 128 * I)
            b = min(hi, 128 * I + 128)
            if a >= b:
                continue
            blk = allowed[a:b, 128 * J:128 * (J + 1)].T   # [j(128), i(b-a)]
            if blk.min() > 0.5:
                continue                                   # all ones: skip
            t = np.zeros((128, 128), np.float32)
            t[:, : b - a] = blk
            mask_apps[J].append((len(tiles), a, b))
            tiles.append(t)
    mask_np = np.stack(tiles, 0) if tiles else np.zeros((1, 128, 128), np.float32)
    return slabs, mask_apps, mask_np


def _split_cols(a, b):
    """Split absolute column range [a,b) at PIECES boundaries (448)."""
    out = []
    for c0, c1 in PIECES:
        s, e = max(a, c0), min(b, c1)
        if s < e:
            out.append((s, e))
    return out


# packed per-partition vectors layout: name -> (col offset, n chunks)
VEC_LAYOUT = {
    "fe_b1": (0, 8), "fe_b2": (8, 4), "fe_g": (12, 4), "fe_beta": (16, 4),
    "gate_b": (20, 4), "fc1_b": (24, 2), "ln_g": (26, 2), "ln_b": (28, 2),
}
NV = 30


def _pack_vecs(inputs):
    v = np.zeros((128, NV), np.float32)
    for name, (off, nch) in VEC_LAYOUT.items():
        arr = np.asarray(inputs[name], np.float32).reshape(nch, 128)
        v[:, off:off + nch] = arr.T
    return v


# ------------------------------------------------------------------- builder --
def _build(n_mask_tiles):
    nc = bacc.Bacc()
    slabs, mask_apps, _ = _mask_meta()

    x_e = nc.declare_dram_parameter("x", [BPC, D, NTP], BF16, isOutput=False)
    w1_e = nc.declare_dram_parameter("fe_w1", [D, DH], BF16, isOutput=False)
    w2_e = nc.declare_dram_parameter("fe_w2", [DH, D], BF16, isOutput=False)
    wq_e = nc.declare_dram_parameter("wq", [D, D], BF16, isOutput=False)
    wk_e = nc.declare_dram_parameter("wk", [D, D], BF16, isOutput=False)
    wv_e = nc.declare_dram_parameter("wv", [D, D], BF16, isOutput=False)
    gw_e = nc.declare_dram_parameter("gate_w", [D, D], BF16, isOutput=False)
    f1_e = nc.declare_dram_parameter("fc1_w", [D, DF], BF16, isOutput=False)
    f2_e = nc.declare_dram_parameter("fc2_w", [DF, OUT], BF16, isOutput=False)
    vec_e = nc.declare_dram_parameter("vecs", [128, NV], F32, isOutput=False)
    b88_e = nc.declare_dram_parameter("b88", [1, OUT], BF16, isOutput=False)
    msk_e = nc.declare_dram_parameter("masks", [n_mask_tiles, 128, 128], BF16,
                                      isOutput=False)
    out_e = nc.declare_dram_parameter("out", [BPC, N, OUT], F32, isOutput=True)

    from contextlib import ExitStack

    with tile.TileContext(nc) as tc:
        with ExitStack() as stk:
            stk.enter_context(
                nc.allow_low_precision(reason="bf16 compute is intended"))
            pool = lambda name, bufs, **kw: stk.enter_context(
                tc.tile_pool(name=name, bufs=bufs, **kw))
            persist = pool("persist", 1)
            w1024 = pool("w1024", 4)
            w512 = pool("w512", 10)
            wsmall = pool("wsmall", 2)
            xin = pool("xin", 5)
            upool = pool("upool", 9)
            ypool = pool("ypool", 5)
            sqp = pool("sq", 3)
            x1p = pool("x1", 5)
            qkp = pool("qk", 5)
            vtmp = pool("vtm", 4)
            pp = pool("pp", 2)
            ofmp = pool("ofm", 5)
            statp = pool("stat", 6)
            predp = pool("pred", 3)
            psA = pool("psA", 4, space="PSUM")
            psB = pool("psB", 2, space="PSUM")
            psC = pool("psC", 2, space="PSUM")

            # constants
            ones_col = persist.tile([128, 1], BF16, name="ones_col")
            nc.vector.memset(ones_col, 1.0)
            ones_row = persist.tile([1, 128], BF16, name="ones_row")
            nc.vector.memset(ones_row, 1.0)
            zero_col = persist.tile([128, 1], F32, name="zero_col")
            nc.vector.memset(zero_col, 0.0)
            eps1 = persist.tile([1, 1], F32, name="eps1")
            nc.vector.memset(eps1, EPS)
            vecs = persist.tile([128, NV], F32, name="vecs")
            nc.sync.dma_start(out=vecs, in_=vec_e[:, :])
            b88 = persist.tile([1, OUT], BF16, name="b88")
            nc.sync.dma_start(out=b88, in_=b88_e[:, :])
            masks = persist.tile([128, n_mask_tiles * 128], BF16, name="masks")
            for mi in range(n_mask_tiles):
                nc.sync.dma_start(
                    out=masks[:, 128 * mi:128 * (mi + 1)], in_=msk_e[mi, :, :])

            def vslice(name, c):
                off, nch = VEC_LAYOUT[name]
                assert c < nch
                return vecs[:, off + c:off + c + 1]

            def mm(ps, lhsT, rhs, start, stop):
                nc.tensor.matmul(ps, lhsT, rhs, start=start, stop=stop)

            for b in range(BPC):
                # ---- load x (feature-major) and FE weights ----
                xt = []
                for c in range(4):
                    t = xin.tile([128, NTP], BF16, tag="xin", name="xin")
                    nc.sync.dma_start(out=t, in_=x_e[b, 128 * c:128 * (c + 1), :])
                    xt.append(t)
                w1t = []
                for c in range(4):
                    t = w1024.tile([128, DH], BF16, tag="w1", name="w1")
                    nc.sync.dma_start(out=t, in_=w1_e[128 * c:128 * (c + 1), :])
                    w1t.append(t)
                w2t = []
                for c in range(8):
                    t = w512.tile([128, D], BF16, tag="w512", name="w512")
                    nc.sync.dma_start(out=t, in_=w2_e[128 * c:128 * (c + 1), :])
                    w2t.append(t)

                # ---- FE block + LN1 -> x1 (feature-major [4][128, NTP]) ----
                yt = [ypool.tile([128, NTP], BF16, tag="y", name="y")
                      for _ in range(4)]
                x1t = [x1p.tile([128, NTP], BF16, tag="x1", name="x1")
                       for _ in range(4)]
                for (c0, c1) in PIECES:
                    w = c1 - c0
                    # u = gelu(x @ w1 + b1)  (8 chunks of 128 features)
                    ut = []
                    for fo in range(8):
                        ps = psA.tile([128, w], F32, tag="psA", name="psA")
                        for fi in range(4):
                            mm(ps, w1t[fi][:, 128 * fo:128 * (fo + 1)],
                               xt[fi][:, c0:c1], fi == 0, fi == 3)
                        t = upool.tile([128, 448], BF16, tag="u", name="u")
                        nc.scalar.activation(t[:, :w], ps, AF.Gelu,
                                             bias=vslice("fe_b1", fo))
                        ut.append(t)
                    # y = u @ w2 + b2 + x
                    for fo in range(4):
                        ps = psA.tile([128, w], F32, tag="psA", name="psA")
                        for fi in range(8):
                            mm(ps, w2t[fi][:, 128 * fo:128 * (fo + 1)],
                               ut[fi][:, :w], fi == 0, fi == 7)
                        nc.scalar.activation(yt[fo][:, c0:c1], ps, AF.Identity,
                                             bias=vslice("fe_b2", fo))
                        nc.vector.tensor_tensor(yt[fo][:, c0:c1], yt[fo][:, c0:c1],
                                                xt[fo][:, c0:c1], op=OP.add)
                    # LN1 stats over 512 features (ones-matmul on PE)
                    sum_ps = psB.tile([1, w], F32, tag="psB", name="psB")
                    sq_ps = psB.tile([1, w], F32, tag="psB", name="psB")
                    for c in range(4):
                        mm(sum_ps, ones_col, yt[c][:, c0:c1], c == 0, c == 3)
                    for c in range(4):
                        sqt = sqp.tile([128, 448], BF16, tag="sq", name="sq")
                        nc.vector.tensor_tensor(sqt[:, :w], yt[c][:, c0:c1],
                                                yt[c][:, c0:c1], op=OP.mult)
                        mm(sq_ps, ones_col, sqt[:, :w], c == 0, c == 3)
                    mu = statp.tile([1, 448], BF16, tag="statb", name="statb")
                    r = statp.tile([1, 448], BF16, tag="statb", name="statb")
                    ey2 = statp.tile([1, 448], F32, tag="statf", name="statf")
                    var = statp.tile([1, 448], F32, tag="statf", name="statf")
                    nc.scalar.activation(mu[:, :w], sum_ps, AF.Copy, scale=1.0 / D)
                    nc.scalar.activation(ey2[:, :w], sq_ps, AF.Copy, scale=1.0 / D)
                    nc.vector.tensor_tensor(var[:, :w], mu[:, :w], mu[:, :w],
                                            op=OP.mult)
                    nc.vector.tensor_tensor(var[:, :w], ey2[:, :w], var[:, :w],
                                            op=OP.subtract)
                    nc.scalar.activation(var[:, :w], var[:, :w], AF.Sqrt,
                                         bias=eps1)
                    nc.vector.reciprocal(r[:, :w], var[:, :w])
                    mu_b = psC.tile([128, w], F32, tag="psC", name="psC")
                    r_b = psC.tile([128, w], F32, tag="psC", name="psC")
                    mm(mu_b, ones_row, mu[:, :w], True, True)
                    mm(r_b, ones_row, r[:, :w], True, True)
                    for c in range(4):
                        nc.vector.tensor_tensor(x1t[c][:, c0:c1], yt[c][:, c0:c1],
                                                mu_b, op=OP.subtract)
                        nc.vector.tensor_tensor(x1t[c][:, c0:c1], x1t[c][:, c0:c1],
                                                r_b, op=OP.mult)
                        nc.vector.tensor_scalar(
                            out=x1t[c][:, c0:c1], in0=x1t[c][:, c0:c1],
                            scalar1=vslice("fe_g", c), scalar2=vslice("fe_beta", c),
                            op0=OP.mult, op1=OP.add)

                # ---- QKV ----
                wqt, wkt, wvt = [], [], []
                for we, lst in ((wq_e, wqt), (wk_e, wkt), (wv_e, wvt)):
                    for c in range(4):
                        t = w512.tile([128, D], BF16, tag="w512", name="w512")
                        nc.sync.dma_start(out=t, in_=we[128 * c:128 * (c + 1), :])
                        lst.append(t)
                qt, kt = [], []
                for h in range(H):
                    qh = qkp.tile([128, NTP], BF16, tag="q", name="q")
                    for (c0, c1) in PIECES:
                        ps = psA.tile([128, c1 - c0], F32, tag="psA", name="psA")
                        for fi in range(4):
                            mm(ps, wqt[fi][:, 128 * h:128 * (h + 1)],
                               x1t[fi][:, c0:c1], fi == 0, fi == 3)
                        nc.scalar.activation(qh[:, c0:c1], ps, AF.Copy)
                    qt.append(qh)
                    kh = qkp.tile([128, NK], BF16, tag="k", name="k", bufs=4)
                    ps = psA.tile([128, NK], F32, tag="psA", name="psA")
                    for fi in range(4):
                        mm(ps, wkt[fi][:, 128 * h:128 * (h + 1)],
                           x1t[fi][:, 0:NK], fi == 0, fi == 3)
                    nc.scalar.activation(kh, ps, AF.Copy)
                    kt.append(kh)
                vtm = []
                for tch in range(NKC):
                    ps = psA.tile([128, D], F32, tag="psA", name="psA")
                    for fi in range(4):
                        mm(ps, x1t[fi][:, 128 * tch:128 * (tch + 1)],
                           wvt[fi], fi == 0, fi == 3)
                    t = vtmp.tile([128, D], BF16, tag="vtm", name="vtm")
                    nc.vector.tensor_copy(out=t, in_=ps)
                    vtm.append(t)

                # ---- attention (per head) ----
                ot = [ofmp.tile([128, NTP], BF16, tag="ofm", name="ofm")
                      for _ in range(H)]
                for h in range(H):
                    pt = {}
                    for J in range(NKC):
                        lo, hi = slabs[J]
                        p = pp.tile([128, hi - lo], BF16, tag=f"p{J}",
                                    name=f"p{J}")
                        for (a, e) in _split_cols(lo, hi):
                            ps = psA.tile([128, e - a], F32, tag="psA",
                                          name="psA")
                            mm(ps, kt[h][:, 128 * J:128 * (J + 1)], qt[h][:, a:e],
                               True, True)
                            nc.scalar.activation(p[:, a - lo:e - lo], ps, AF.Exp,
                                                 bias=zero_col, scale=SCALE)
                        for (mi, a, e) in mask_apps[J]:
                            nc.vector.tensor_tensor(
                                p[:, a - lo:e - lo], p[:, a - lo:e - lo],
                                masks[:, 128 * mi:128 * mi + (e - a)], op=OP.mult)
                        pt[J] = p
                    for (A, Bc) in PIECES:
                        w = Bc - A
                        den = psB.tile([1, w], F32, tag="psB", name="psB")
                        cover = []
                        for J in range(NKC):
                            lo, hi = slabs[J]
                            s, e = max(lo, A), min(hi, Bc)
                            if s < e:
                                cover.append((J, s, e))
                        for idx, (J, s, e) in enumerate(cover):
                            lo = slabs[J][0]
                            mm(den[:, s - A:e - A], ones_col,
                               pt[J][:, s - lo:e - lo],
                               idx == 0, idx == len(cover) - 1)
                        rden = statp.tile([1, 448], BF16, tag="statb",
                                          name="statb")
                        nc.vector.reciprocal(rden[:, :w], den)
                        rb = psC.tile([128, w], F32, tag="psC", name="psC")
                        mm(rb, ones_row, rden[:, :w], True, True)
                        for (J, s, e) in cover:
                            lo = slabs[J][0]
                            nc.vector.tensor_tensor(
                                pt[J][:, s - lo:e - lo], pt[J][:, s - lo:e - lo],
                                rb[:, s - A:e - A], op=OP.mult)
                        ops = psA.tile([128, w], F32, tag="psA", name="psA")
                        for idx, (J, s, e) in enumerate(cover):
                            lo = slabs[J][0]
                            mm(ops[:, s - A:e - A],
                               vtm[J][:, 128 * h:128 * (h + 1)],
                               pt[J][:, s - lo:e - lo],
                               idx == 0, idx == len(cover) - 1)
                        nc.scalar.activation(ot[h][:, A:Bc], ops, AF.Copy)

                # ---- gate (in-place: ot *= sigmoid(ot @ gate_w + b)) ----
                gwt = []
                for c in range(4):
                    t = w512.tile([128, D], BF16, tag="w512", name="w512")
                    nc.sync.dma_start(out=t, in_=gw_e[128 * c:128 * (c + 1), :])
                    gwt.append(t)
                for (c0, c1) in PIECES:
                    w = c1 - c0
                    for fo in range(4):
                        ps = psA.tile([128, w], F32, tag="psA", name="psA")
                        for fi in range(4):
                            mm(ps, gwt[fi][:, 128 * fo:128 * (fo + 1)],
                               ot[fi][:, c0:c1], fi == 0, fi == 3)
                        g = sqp.tile([128, 448], BF16, tag="sq", name="sq")
                        nc.scalar.activation(g[:, :w], ps, AF.Sigmoid,
                                             bias=vslice("gate_b", fo))
                        nc.vector.tensor_tensor(ot[fo][:, c0:c1], ot[fo][:, c0:c1],
                                                g[:, :w], op=OP.mult)

                # ---- fc1 + LN2 + gelu ----
                f1t = []
                for c in range(4):
                    t = wsmall.tile([128, DF], BF16, tag="wf1", name="wf1",
                                    bufs=4)
                    nc.sync.dma_start(out=t, in_=f1_e[128 * c:128 * (c + 1), :])
                    f1t.append(t)
                f2t = []
                for c in range(2):
                    t = wsmall.tile([128, OUT], BF16, tag="wf2", name="wf2")
                    nc.sync.dma_start(out=t, in_=f2_e[128 * c:128 * (c + 1), :])
                    f2t.append(t)
                h2 = [x1p.tile([128, NTP], BF16, tag="x1", name="x1")
                      for _ in range(2)]
                tt = [ypool.tile([128, NTP], BF16, tag="y", name="y")
                      for _ in range(2)]
                for (c0, c1) in PIECES:
                    w = c1 - c0
                    for fo in range(2):
                        ps = psA.tile([128, w], F32, tag="psA", name="psA")
                        for fi in range(4):
                            mm(ps, f1t[fi][:, 128 * fo:128 * (fo + 1)],
                               ot[fi][:, c0:c1], fi == 0, fi == 3)
                        nc.scalar.activation(tt[fo][:, c0:c1], ps, AF.Identity,
                                             bias=vslice("fc1_b", fo))
                    sum_ps = psB.tile([1, w], F32, tag="psB", name="psB")
                    sq_ps = psB.tile([1, w], F32, tag="psB", name="psB")
                    for c in range(2):
                        mm(sum_ps, ones_col, tt[c][:, c0:c1], c == 0, c == 1)
                    for c in range(2):
                        sqt = sqp.tile([128, 448], BF16, tag="sq", name="sq")
                        nc.vector.tensor_tensor(sqt[:, :w], tt[c][:, c0:c1],
                                                tt[c][:, c0:c1], op=OP.mult)
                        mm(sq_ps, ones_col, sqt[:, :w], c == 0, c == 1)
                    mu = statp.tile([1, 448], BF16, tag="statb", name="statb")
                    r = statp.tile([1, 448], BF16, tag="statb", name="statb")
                    ey2 = statp.tile([1, 448], F32, tag="statf", name="statf")
                    var = statp.tile([1, 448], F32, tag="statf", name="statf")
                    nc.scalar.activation(mu[:, :w], sum_ps, AF.Copy, scale=1.0 / DF)
                    nc.scalar.activation(ey2[:, :w], sq_ps, AF.Copy, scale=1.0 / DF)
                    nc.vector.tensor_tensor(var[:, :w], mu[:, :w], mu[:, :w],
                                            op=OP.mult)
                    nc.vector.tensor_tensor(var[:, :w], ey2[:, :w], var[:, :w],
                                            op=OP.subtract)
                    nc.scalar.activation(var[:, :w], var[:, :w], AF.Sqrt,
                                         bias=eps1)
                    nc.vector.reciprocal(r[:, :w], var[:, :w])
                    mu_b = psC.tile([128, w], F32, tag="psC", name="psC")
                    r_b = psC.tile([128, w], F32, tag="psC", name="psC")
                    mm(mu_b, ones_row, mu[:, :w], True, True)
                    mm(r_b, ones_row, r[:, :w], True, True)
                    for c in range(2):
                        tn = sqp.tile([128, 448], BF16, tag="sq", name="sq")
                        nc.vector.tensor_tensor(tn[:, :w], tt[c][:, c0:c1], mu_b,
                                                op=OP.subtract)
                        nc.vector.tensor_tensor(tn[:, :w], tn[:, :w], r_b,
                                                op=OP.mult)
                        nc.vector.tensor_scalar(
                            out=tn[:, :w], in0=tn[:, :w],
                            scalar1=vslice("ln_g", c), scalar2=vslice("ln_b", c),
                            op0=OP.mult, op1=OP.add)
                        nc.scalar.activation(h2[c][:, c0:c1], tn[:, :w], AF.Gelu,
                                             bias=zero_col)

                # ---- fc2 (token-major) + sigmoid + store ----
                for tch in range(NC_TOK):
                    ps = psA.tile([128, OUT], F32, tag="psA", name="psA")
                    for fi in range(2):
                        mm(ps, h2[fi][:, 128 * tch:128 * (tch + 1)], f2t[fi],
                           fi == 0, False)
                    mm(ps, ones_row, b88, False, True)
                    pr = predp.tile([128, OUT], F32, tag="pred", name="pred")
                    nc.scalar.activation(pr, ps, AF.Sigmoid, bias=zero_col)
                    nc.sync.dma_start(
                        out=out_e[b, 128 * tch:128 * (tch + 1), :], in_=pr)
    nc.finalize()
    return nc


# -------------------------------------------------------------------- driver --
_CACHE = {}
LAST_RESULT = None


def kernel(**inputs):
    global LAST_RESULT
    inputs = {k: np.asarray(v, np.float32) for k, v in inputs.items()}
    slabs, mask_apps, mask_np = _mask_meta()
    if "nc" not in _CACHE:
        _CACHE["nc"] = _build(mask_np.shape[0])
    nc = _CACHE["nc"]

    vecs = _pack_vecs(inputs)

    def bf(a):
        return np.ascontiguousarray(np.asarray(a, np.float32).astype(NPBF))

    shared = {
        "fe_w1": bf(inputs["fe_w1"]), "fe_w2": bf(inputs["fe_w2"]),
        "wq": bf(inputs["wq"]), "wk": bf(inputs["wk"]), "wv": bf(inputs["wv"]),
        "gate_w": bf(inputs["gate_w"]), "fc1_w": bf(inputs["fc1_w"]),
        "fc2_w": bf(inputs["fc2_w"]), "vecs": np.ascontiguousarray(vecs),
        "b88": bf(inputs["fc2_b"].reshape(1, OUT)), "masks": bf(mask_np),
    }
    in_maps = []
    for c in range(NCORES):
        xs = inputs["x"][c * BPC:(c + 1) * BPC]                # [BPC, N, D]
        x_fm = bf(xs.transpose(0, 2, 1)[:, :, :NTP])            # [BPC, D, NTP]
        in_maps.append({"x": x_fm, **shared})

    res = run_bass_kernel_spmd(nc, in_maps, core_ids=list(range(NCORES)))
    LAST_RESULT = res
    out = np.concatenate([res.results[i]["out"] for i in range(NCORES)], axis=0)
    out = np.asarray(out, np.float32).copy()
    out[:, NT:, :] = np.nan
    return out


# revision 13
# speedup vs baseline: 1.0704x; 1.0704x over previous
"""Trainium2 Bass kernel for the AdvancedMambaAMT block (sparse attention).

Strategy:
  - Data-parallel over batch: 16 batches / 8 cores = 2 batches per core.
    No collectives needed; host shards inputs and concatenates outputs.
  - The causal*decay mask, under XLA's flush-to-zero float32 semantics,
    is exactly: allowed(i,j) = (j <= i) AND (i + j <= 873).
    Queries i >= 874 have NO allowed keys -> reference produces NaN rows;
    we reproduce those NaNs host-side.  Keys j >= 437 are never attended.
  - All activations are kept feature-major [feature_chunk(128), token] so
    the whole matmul chain needs zero on-device transposes; `x` is
    pre-transposed on the host.  LayerNorm statistics (partition-axis
    reductions) are computed with ones-vector matmuls on the PE, and
    per-token scalars are broadcast across partitions with K=1 matmuls.
  - Attention is computed in [key, query] orientation: scoresT = k^T q,
    p = exp(scale*scoresT) * mask01, den = ones^T p, out = v^T (p/den).
    Only the 4 non-empty key chunks and their exact query extents are
    computed (~18% of the dense score matrix).
  - Matmul operands are bf16 (1 PE cycle/row vs fp32's 4); accumulation
    stays fp32 in PSUM, softmax/LN statistics stay fp32.
"""

import os
import sys

os.environ.setdefault("MYCRO_LOCAL_CACHE", "1")
if "/opt/trn_rl_repo" not in sys.path:
    sys.path.insert(0, "/opt/trn_rl_repo")

import ml_dtypes
import numpy as np

import concourse.bass as bass
import concourse.mybir as mybir
import concourse.tile as tile
from concourse import bacc
from concourse.bass_utils import run_bass_kernel_spmd

F32 = mybir.dt.float32
BF16 = mybir.dt.bfloat16
AF = mybir.ActivationFunctionType
OP = mybir.AluOpType
NPBF = ml_dtypes.bfloat16

B, N, D = 16, 1024, 512
H, HD, OUT = 4, 128, 88
DH = 2 * D          # FE hidden 1024
DF = D // 2         # fc1 out 256
EPS = 1e-5
SCALE = 1.0 / float(np.sqrt(HD))

THR = 873           # allowed(i,j) = (j<=i) & (i+j<=THR); verified vs jax in test.py
NT = THR + 1        # 874 valid query tokens
NTP = 896           # 7*128, padded token count we compute
NK = 512            # key tokens used (4 chunks; real max key is 436)
NCORES = 8
BPC = B // NCORES   # batches per core

PIECES = [(0, 448), (448, NTP)]          # column pieces (PSUM-bank sized)
NC_TOK = NTP // 128                      # 7 token chunks
NKC = NK // 128                          # 4 key chunks


# ---------------------------------------------------------------- host mask --
def _mask_meta():
    """Slab extents, mask-multiply schedule, and packed mask tiles.

    Returns (slabs, mask_apps, mask_np):
      slabs[J] = (lo, hi) query-column range computed for key chunk J
      mask_apps[J] = list of (tile_idx, a, b): p[:, a-lo:b-lo] *= mask[ti][:, :b-a]
      mask_np = [n_tiles, 128, 128] float32 0/1 tiles in [j, i] orientation
    """
    i = np.arange(N)[:, None]
    j = np.arange(N)[None, :]
    allowed = ((j <= i) & (i + j <= THR)).astype(np.float32)  # [i, j]
    slabs = []
    for J in range(NKC):
        lo = 128 * J
        hi = THR - 128 * J + 1
        if J == 0:
            hi = NTP                      # force full PSUM-column initialization
        slabs.append((lo, hi))
    tiles = []
    mask_apps = [[] for _ in range(NKC)]
    for J in range(NKC):
        lo, hi = slabs[J]
        for I in range((NTP + 127) // 128):
            a = max(lo, 128 * I)
            b = min(hi, 128 * I + 128)
            if a >= b:
                continue
            blk = allowed[a:b, 128 * J:128 * (J + 1)].T   # [j(128), i(b-a)]
            if blk.min() > 0.5:
                continue                                   # all ones: skip
            t = np.zeros((128, 128), np.float32)
            t[:, : b - a] = blk
            mask_apps[J].append((len(tiles), a, b))
            tiles.append(t)
    mask_np = np.stack(tiles, 0) if tiles else np.zeros((1, 128, 128), np.float32)
    return slabs, mask_apps, mask_np


def _split_cols(a, b):
    """Split absolute column range [a,b) at PIECES boundaries (448)."""
    out = []
    for c0, c1 in PIECES:
        s, e = max(a, c0), min(b, c1)
        if s < e:
            out.append((s, e))
    return out


# packed per-partition vectors layout: name -> (col offset, n chunks)
VEC_LAYOUT = {
    "fe_b1": (0, 8), "fe_b2": (8, 4), "fe_g": (12, 4), "fe_beta": (16, 4),
    "gate_b": (20, 4), "fc1_b": (24, 2), "ln_g": (26, 2), "ln_b": (28, 2),
}
NV = 30


def _pack_vecs(inputs):
    v = np.zeros((128, NV), np.float32)
    for name, (off, nch) in VEC_LAYOUT.items():
        arr = np.asarray(inputs[name], np.float32).reshape(nch, 128)
        v[:, off:off + nch] = arr.T
    return v


# ------------------------------------------------------------------- builder --
def _build(n_mask_tiles):
    nc = bacc.Bacc()
    slabs, mask_apps, _ = _mask_meta()

    x_e = nc.declare_dram_parameter("x", [BPC, D, NTP], BF16, isOutput=False)
    w1_e = nc.declare_dram_parameter("fe_w1", [D, DH], BF16, isOutput=False)
    w2_e = nc.declare_dram_parameter("fe_w2", [DH, D], BF16, isOutput=False)
    wq_e = nc.declare_dram_parameter("wq", [D, D], BF16, isOutput=False)
    wk_e = nc.declare_dram_parameter("wk", [D, D], BF16, isOutput=False)
    wv_e = nc.declare_dram_parameter("wv", [D, D], BF16, isOutput=False)
    gw_e = nc.declare_dram_parameter("gate_w", [D, D], BF16, isOutput=False)
    f1_e = nc.declare_dram_parameter("fc1_w", [D, DF], BF16, isOutput=False)
    f2_e = nc.declare_dram_parameter("fc2_w", [DF, OUT], BF16, isOutput=False)
    vec_e = nc.declare_dram_parameter("vecs", [128, NV], F32, isOutput=False)
    b88_e = nc.declare_dram_parameter("b88", [1, OUT], BF16, isOutput=False)
    msk_e = nc.declare_dram_parameter("masks", [n_mask_tiles, 128, 128], BF16,
                                      isOutput=False)
    out_e = nc.declare_dram_parameter("out", [BPC, N, OUT], F32, isOutput=True)

    from contextlib import ExitStack

    with tile.TileContext(nc) as tc:
        with ExitStack() as stk:
            stk.enter_context(
                nc.allow_low_precision(reason="bf16 compute is intended"))
            pool = lambda name, bufs, **kw: stk.enter_context(
                tc.tile_pool(name=name, bufs=bufs, **kw))
            persist = pool("persist", 1)
            w1024 = pool("w1024", 4)
            w512 = pool("w512", 10)
            wsmall = pool("wsmall", 2)
            xin = pool("xin", 5)
            upool = pool("upool", 9)
            ypool = pool("ypool", 5)
            sqp = pool("sq", 3)
            x1p = pool("x1", 5)
            qkp = pool("qk", 5)
            vtmp = pool("vtm", 4)
            pp = pool("pp", 2)
            ofmp = pool("ofm", 5)
            statp = pool("stat", 6)
            predp = pool("pred", 3)
            bcp = pool("bc", 6)
            psA = pool("psA", 6, space="PSUM")
            psB = pool("psB", 2, space="PSUM")

            # constants
            ones_col = persist.tile([128, 1], BF16, name="ones_col")
            nc.vector.memset(ones_col, 1.0)
            ones_row = persist.tile([1, 128], BF16, name="ones_row")
            nc.vector.memset(ones_row, 1.0)
            zero_col = persist.tile([128, 1], F32, name="zero_col")
            nc.vector.memset(zero_col, 0.0)
            eps1 = persist.tile([1, 1], F32, name="eps1")
            nc.vector.memset(eps1, EPS)
            vecs = persist.tile([128, NV], F32, name="vecs")
            nc.sync.dma_start(out=vecs, in_=vec_e[:, :])
            b88 = persist.tile([1, OUT], BF16, name="b88")
            nc.sync.dma_start(out=b88, in_=b88_e[:, :])
            masks = persist.tile([128, n_mask_tiles * 128], BF16, name="masks")
            for mi in range(n_mask_tiles):
                nc.sync.dma_start(
                    out=masks[:, 128 * mi:128 * (mi + 1)], in_=msk_e[mi, :, :])

            def vslice(name, c):
                off, nch = VEC_LAYOUT[name]
                assert c < nch
                return vecs[:, off + c:off + c + 1]

            def mm(ps, lhsT, rhs, start, stop):
                nc.tensor.matmul(ps, lhsT, rhs, start=start, stop=stop)

            for b in range(BPC):
                # ---- load x (feature-major) and FE weights ----
                xt = []
                for c in range(4):
                    t = xin.tile([128, NTP], BF16, tag="xin", name="xin")
                    nc.sync.dma_start(out=t, in_=x_e[b, 128 * c:128 * (c + 1), :])
                    xt.append(t)
                w1t = []
                for c in range(4):
                    t = w1024.tile([128, DH], BF16, tag="w1", name="w1")
                    nc.sync.dma_start(out=t, in_=w1_e[128 * c:128 * (c + 1), :])
                    w1t.append(t)
                w2t = []
                for c in range(8):
                    t = w512.tile([128, D], BF16, tag="w512", name="w512")
                    nc.sync.dma_start(out=t, in_=w2_e[128 * c:128 * (c + 1), :])
                    w2t.append(t)

                # ---- FE block + LN1 -> x1 (feature-major [4][128, NTP]) ----
                yt = [ypool.tile([128, NTP], BF16, tag="y", name="y")
                      for _ in range(4)]
                x1t = [x1p.tile([128, NTP], BF16, tag="x1", name="x1")
                       for _ in range(4)]
                for (c0, c1) in PIECES:
                    w = c1 - c0
                    # u = gelu(x @ w1 + b1)  (8 chunks of 128 features)
                    ut = []
                    for fo in range(8):
                        ps = psA.tile([128, w], F32, tag="psA", name="psA")
                        for fi in range(4):
                            mm(ps, w1t[fi][:, 128 * fo:128 * (fo + 1)],
                               xt[fi][:, c0:c1], fi == 0, fi == 3)
                        t = upool.tile([128, 448], BF16, tag="u", name="u")
                        nc.scalar.activation(t[:, :w], ps, AF.Gelu,
                                             bias=vslice("fe_b1", fo))
                        ut.append(t)
                    # y = u @ w2 + b2 + x
                    for fo in range(4):
                        ps = psA.tile([128, w], F32, tag="psA", name="psA")
                        for fi in range(8):
                            mm(ps, w2t[fi][:, 128 * fo:128 * (fo + 1)],
                               ut[fi][:, :w], fi == 0, fi == 7)
                        nc.vector.scalar_tensor_tensor(
                            out=yt[fo][:, c0:c1], in0=ps,
                            scalar=vslice("fe_b2", fo), in1=xt[fo][:, c0:c1],
                            op0=OP.add, op1=OP.add)
                    # LN1 stats over 512 features (ones-matmul on PE)
                    sum_ps = psB.tile([1, w], F32, tag="psB", name="psB")
                    sq_ps = psB.tile([1, w], F32, tag="psB", name="psB")
                    for c in range(4):
                        mm(sum_ps, ones_col, yt[c][:, c0:c1], c == 0, c == 3)
                    for c in range(4):
                        sqt = sqp.tile([128, 448], BF16, tag="sq", name="sq")
                        nc.vector.tensor_tensor(sqt[:, :w], yt[c][:, c0:c1],
                                                yt[c][:, c0:c1], op=OP.mult)
                        mm(sq_ps, ones_col, sqt[:, :w], c == 0, c == 3)
                    mu = statp.tile([1, 448], BF16, tag="statb", name="statb")
                    r = statp.tile([1, 448], BF16, tag="statb", name="statb")
                    ey2 = statp.tile([1, 448], F32, tag="statf", name="statf")
                    var = statp.tile([1, 448], F32, tag="statf", name="statf")
                    nc.vector.tensor_scalar_mul(mu[:, :w], sum_ps, 1.0 / D)
                    nc.vector.tensor_scalar_mul(ey2[:, :w], sq_ps, 1.0 / D)
                    nc.vector.tensor_tensor(var[:, :w], mu[:, :w], mu[:, :w],
                                            op=OP.mult)
                    nc.vector.tensor_tensor(var[:, :w], ey2[:, :w], var[:, :w],
                                            op=OP.subtract)
                    nc.scalar.activation(var[:, :w], var[:, :w], AF.Sqrt,
                                         bias=eps1)
                    nc.vector.reciprocal(r[:, :w], var[:, :w])
                    mu_b = bcp.tile([128, 448], BF16, tag="bc", name="bc")
                    r_b = bcp.tile([128, 448], BF16, tag="bc", name="bc")
                    nc.gpsimd.partition_broadcast(mu_b[:, :w], mu[:, :w])
                    nc.gpsimd.partition_broadcast(r_b[:, :w], r[:, :w])
                    for c in range(4):
                        nc.vector.tensor_tensor(x1t[c][:, c0:c1], yt[c][:, c0:c1],
                                                mu_b[:, :w], op=OP.subtract)
                        nc.vector.tensor_tensor(x1t[c][:, c0:c1], x1t[c][:, c0:c1],
                                                r_b[:, :w], op=OP.mult)
                        nc.vector.tensor_scalar(
                            out=x1t[c][:, c0:c1], in0=x1t[c][:, c0:c1],
                            scalar1=vslice("fe_g", c), scalar2=vslice("fe_beta", c),
                            op0=OP.mult, op1=OP.add)

                # ---- QKV ----
                wqt, wkt, wvt = [], [], []
                for we, lst in ((wq_e, wqt), (wk_e, wkt), (wv_e, wvt)):
                    for c in range(4):
                        t = w512.tile([128, D], BF16, tag="w512", name="w512")
                        nc.sync.dma_start(out=t, in_=we[128 * c:128 * (c + 1), :])
                        lst.append(t)
                qt, kt = [], []
                for h in range(H):
                    qh = qkp.tile([128, NTP], BF16, tag="q", name="q")
                    for (c0, c1) in PIECES:
                        ps = psA.tile([128, c1 - c0], F32, tag="psA", name="psA")
                        for fi in range(4):
                            mm(ps, wqt[fi][:, 128 * h:128 * (h + 1)],
                               x1t[fi][:, c0:c1], fi == 0, fi == 3)
                        nc.vector.tensor_copy(out=qh[:, c0:c1], in_=ps)
                    qt.append(qh)
                    kh = qkp.tile([128, NK], BF16, tag="k", name="k", bufs=4)
                    ps = psA.tile([128, NK], F32, tag="psA", name="psA")
                    for fi in range(4):
                        mm(ps, wkt[fi][:, 128 * h:128 * (h + 1)],
                           x1t[fi][:, 0:NK], fi == 0, fi == 3)
                    nc.vector.tensor_copy(out=kh, in_=ps)
                    kt.append(kh)
                vtm = []
                for tch in range(NKC):
                    ps = psA.tile([128, D], F32, tag="psA", name="psA")
                    for fi in range(4):
                        mm(ps, x1t[fi][:, 128 * tch:128 * (tch + 1)],
                           wvt[fi], fi == 0, fi == 3)
                    t = vtmp.tile([128, D], BF16, tag="vtm", name="vtm")
                    nc.vector.tensor_copy(out=t, in_=ps)
                    vtm.append(t)

                # ---- attention (per head) ----
                ot = [ofmp.tile([128, NTP], BF16, tag="ofm", name="ofm")
                      for _ in range(H)]
                for h in range(H):
                    pt = {}
                    for J in range(NKC):
                        lo, hi = slabs[J]
                        p = pp.tile([128, hi - lo], BF16, tag=f"p{J}",
                                    name=f"p{J}")
                        for (a, e) in _split_cols(lo, hi):
                            ps = psA.tile([128, e - a], F32, tag="psA",
                                          name="psA")
                            mm(ps, kt[h][:, 128 * J:128 * (J + 1)], qt[h][:, a:e],
                               True, True)
                            nc.scalar.activation(p[:, a - lo:e - lo], ps, AF.Exp,
                                                 bias=zero_col, scale=SCALE)
                        for (mi, a, e) in mask_apps[J]:
                            nc.vector.tensor_tensor(
                                p[:, a - lo:e - lo], p[:, a - lo:e - lo],
                                masks[:, 128 * mi:128 * mi + (e - a)], op=OP.mult)
                        pt[J] = p
                    for (A, Bc) in PIECES:
                        w = Bc - A
                        den = psB.tile([1, w], F32, tag="psB", name="psB")
                        cover = []
                        for J in range(NKC):
                            lo, hi = slabs[J]
                            s, e = max(lo, A), min(hi, Bc)
                            if s < e:
                                cover.append((J, s, e))
                        for idx, (J, s, e) in enumerate(cover):
                            lo = slabs[J][0]
                            mm(den[:, s - A:e - A], ones_col,
                               pt[J][:, s - lo:e - lo],
                               idx == 0, idx == len(cover) - 1)
                        rden = statp.tile([1, 448], BF16, tag="statb",
                                          name="statb")
                        nc.vector.reciprocal(rden[:, :w], den)
                        rb = bcp.tile([128, 448], BF16, tag="bc", name="bc")
                        nc.gpsimd.partition_broadcast(rb[:, :w], rden[:, :w])
                        ops = psA.tile([128, w], F32, tag="psA", name="psA")
                        for idx, (J, s, e) in enumerate(cover):
                            lo = slabs[J][0]
                            mm(ops[:, s - A:e - A],
                               vtm[J][:, 128 * h:128 * (h + 1)],
                               pt[J][:, s - lo:e - lo],
                               idx == 0, idx == len(cover) - 1)
                        nc.vector.tensor_tensor(ot[h][:, A:Bc], ops, rb[:, :w],
                                                op=OP.mult)

                # ---- gate (in-place: ot *= sigmoid(ot @ gate_w + b)) ----
                gwt = []
                for c in range(4):
                    t = w512.tile([128, D], BF16, tag="w512", name="w512")
                    nc.sync.dma_start(out=t, in_=gw_e[128 * c:128 * (c + 1), :])
                    gwt.append(t)
                for (c0, c1) in PIECES:
                    w = c1 - c0
                    for fo in range(4):
                        ps = psA.tile([128, w], F32, tag="psA", name="psA")
                        for fi in range(4):
                            mm(ps, gwt[fi][:, 128 * fo:128 * (fo + 1)],
                               ot[fi][:, c0:c1], fi == 0, fi == 3)
                        g = sqp.tile([128, 448], BF16, tag="sq", name="sq")
                        nc.scalar.activation(g[:, :w], ps, AF.Sigmoid,
                                             bias=vslice("gate_b", fo))
                        nc.vector.tensor_tensor(ot[fo][:, c0:c1], ot[fo][:, c0:c1],
                                                g[:, :w], op=OP.mult)

                # ---- fc1 + LN2 + gelu ----
                f1t = []
                for c in range(4):
                    t = wsmall.tile([128, DF], BF16, tag="wf1", name="wf1",
                                    bufs=4)
                    nc.sync.dma_start(out=t, in_=f1_e[128 * c:128 * (c + 1), :])
                    f1t.append(t)
                f2t = []
                for c in range(2):
                    t = wsmall.tile([128, OUT], BF16, tag="wf2", name="wf2")
                    nc.sync.dma_start(out=t, in_=f2_e[128 * c:128 * (c + 1), :])
                    f2t.append(t)
                h2 = [x1p.tile([128, NTP], BF16, tag="x1", name="x1")
                      for _ in range(2)]
                tt = [ypool.tile([128, NTP], BF16, tag="y", name="y")
                      for _ in range(2)]
                for (c0, c1) in PIECES:
                    w = c1 - c0
                    for fo in range(2):
                        ps = psA.tile([128, w], F32, tag="psA", name="psA")
                        for fi in range(4):
                            mm(ps, f1t[fi][:, 128 * fo:128 * (fo + 1)],
                               ot[fi][:, c0:c1], fi == 0, fi == 3)
                        nc.vector.tensor_scalar_add(tt[fo][:, c0:c1], ps,
                                                    vslice("fc1_b", fo))
                    sum_ps = psB.tile([1, w], F32, tag="psB", name="psB")
                    sq_ps = psB.tile([1, w], F32, tag="psB", name="psB")
                    for c in range(2):
                        mm(sum_ps, ones_col, tt[c][:, c0:c1], c == 0, c == 1)
                    for c in range(2):
                        sqt = sqp.tile([128, 448], BF16, tag="sq", name="sq")
                        nc.vector.tensor_tensor(sqt[:, :w], tt[c][:, c0:c1],
                                                tt[c][:, c0:c1], op=OP.mult)
                        mm(sq_ps, ones_col, sqt[:, :w], c == 0, c == 1)
                    mu = statp.tile([1, 448], BF16, tag="statb", name="statb")
                    r = statp.tile([1, 448], BF16, tag="statb", name="statb")
                    ey2 = statp.tile([1, 448], F32, tag="statf", name="statf")
                    var = statp.tile([1, 448], F32, tag="statf", name="statf")
                    nc.vector.tensor_scalar_mul(mu[:, :w], sum_ps, 1.0 / DF)
                    nc.vector.tensor_scalar_mul(ey2[:, :w], sq_ps, 1.0 / DF)
                    nc.vector.tensor_tensor(var[:, :w], mu[:, :w], mu[:, :w],
                                            op=OP.mult)
                    nc.vector.tensor_tensor(var[:, :w], ey2[:, :w], var[:, :w],
                                            op=OP.subtract)
                    nc.scalar.activation(var[:, :w], var[:, :w], AF.Sqrt,
                                         bias=eps1)
                    nc.vector.reciprocal(r[:, :w], var[:, :w])
                    mu_b = bcp.tile([128, 448], BF16, tag="bc", name="bc")
                    r_b = bcp.tile([128, 448], BF16, tag="bc", name="bc")
                    nc.gpsimd.partition_broadcast(mu_b[:, :w], mu[:, :w])
                    nc.gpsimd.partition_broadcast(r_b[:, :w], r[:, :w])
                    for c in range(2):
                        tn = sqp.tile([128, 448], BF16, tag="sq", name="sq")
                        nc.vector.tensor_tensor(tn[:, :w], tt[c][:, c0:c1],
                                                mu_b[:, :w], op=OP.subtract)
                        nc.vector.tensor_tensor(tn[:, :w], tn[:, :w], r_b[:, :w],
                                                op=OP.mult)
                        nc.vector.tensor_scalar(
                            out=tn[:, :w], in0=tn[:, :w],
                            scalar1=vslice("ln_g", c), scalar2=vslice("ln_b", c),
                            op0=OP.mult, op1=OP.add)
                        nc.scalar.activation(h2[c][:, c0:c1], tn[:, :w], AF.Gelu,
                                             bias=zero_col)

                # ---- fc2 (token-major) + sigmoid + store ----
                for tch in range(NC_TOK):
                    ps = psA.tile([128, OUT], F32, tag="psA", name="psA")
                    for fi in range(2):
                        mm(ps, h2[fi][:, 128 * tch:128 * (tch + 1)], f2t[fi],
                           fi == 0, False)
                    mm(ps, ones_row, b88, False, True)
                    pr = predp.tile([128, OUT], F32, tag="pred", name="pred")
                    nc.scalar.activation(pr, ps, AF.Sigmoid, bias=zero_col)
                    nc.sync.dma_start(
                        out=out_e[b, 128 * tch:128 * (tch + 1), :], in_=pr)
    nc.finalize()
    return nc


# -------------------------------------------------------------------- driver --
_CACHE = {}
LAST_RESULT = None


def kernel(**inputs):
    global LAST_RESULT
    inputs = {k: np.asarray(v, np.float32) for k, v in inputs.items()}
    slabs, mask_apps, mask_np = _mask_meta()
    if "nc" not in _CACHE:
        _CACHE["nc"] = _build(mask_np.shape[0])
    nc = _CACHE["nc"]

    vecs = _pack_vecs(inputs)

    def bf(a):
        return np.ascontiguousarray(np.asarray(a, np.float32).astype(NPBF))

    shared = {
        "fe_w1": bf(inputs["fe_w1"]), "fe_w2": bf(inputs["fe_w2"]),
        "wq": bf(inputs["wq"]), "wk": bf(inputs["wk"]), "wv": bf(inputs["wv"]),
        "gate_w": bf(inputs["gate_w"]), "fc1_w": bf(inputs["fc1_w"]),
        "fc2_w": bf(inputs["fc2_w"]), "vecs": np.ascontiguousarray(vecs),
        "b88": bf(inputs["fc2_b"].reshape(1, OUT)), "masks": bf(mask_np),
    }
    in_maps = []
    for c in range(NCORES):
        xs = inputs["x"][c * BPC:(c + 1) * BPC]                # [BPC, N, D]
        x_fm = bf(xs.transpose(0, 2, 1)[:, :, :NTP])            # [BPC, D, NTP]
        in_maps.append({"x": x_fm, **shared})

    res = run_bass_kernel_spmd(nc, in_maps, core_ids=list(range(NCORES)))
    LAST_RESULT = res
    out = np.concatenate([res.results[i]["out"] for i in range(NCORES)], axis=0)
    out = np.asarray(out, np.float32).copy()
    out[:, NT:, :] = np.nan
    return out
